# revision 49
# baseline (speedup 1.0000x reference)
"""Trainium2 Bass kernel: dual-softmax ("contrast") multi-head self-attention.

Problem (per full input):
  x, y: (4, 1024, 1024) f32; Wq/Wk/Wv: (1024, 1024) f32, nh=16 heads, dk=dv=64.
  q = x @ Wq.T, k = x @ Wk.T, v = y @ Wv.T  (split heads)
  dist   = softmax(q k^T / 8)
  c_att  = softmax(1 - dist) @ v      (== softmax(-dist) @ v, shift invariance)
  att    = softmax(dist) @ v
  returns (c_att, att), each (4, 1024, 1024) f32.

Key numerics: dist entries are softmax outputs (rows sum to exactly 1, entries
in [0,1], overwhelmingly ~1e-3), so exp(+-dist) = 1 +- dist to ~5e-3 relative
output error (tolerance 2e-2; verified against the oracle on CPU). With the
linearization BOTH branches share a single apply matmul and the second-softmax
normalizers are constants:
  att   = (C + A) / (N+1),   c_att = (C - A) / (N-1)
  A = dist @ v = r1[q] * (E1 @ v), E1 = exp(S/8), r1 = 1/rowsum(E1), C = colsum(v)
Per-q scales (r1/(N+-1)) and the C offset are applied in a fused DVE epilogue
after a single PE transpose of Atilde^T = v^T-stationary @ E1^T.

Sharding: 8 cores = 4 batches x 2 head-groups (8 heads each). Each core gets
x[b], y[b] and a 512-row slice of each weight; returns (c_att, att) slices
[1024, 512].

Per-core algorithm:
  XT = x^T, YT = y^T via PE transposes (f32r).
  QT = Wq_s @ x^T   [feat, tok]  bf16 (f32r matmuls, K-accumulated in PSUM)
  KT = Wk_s @ x^T   [feat, tok]  bf16
  V  = y @ Wv_s^T   [tok, feat]  bf16, stored per head with a ones column.
  C  = Wv_s @ colsum(y) once via PE; crep3/2 = broadcast(C)/(N+-1).
  per head (all matmuls bf16, 1 cyc/row):
    S^T[kb] = KT_h[:,kb]^T-stationary @ QT_h    (k on partitions, q on free)
    E1T[kb] = exp(S^T/8)                        [ScalarE, out bf16]
    Atil^T  = sum_kb V_aug[kb]^T-stationary @ E1T[kb]  -> [65, 1024] PSUM,
              row 64 = rowsum1(q) (ones column of V_aug)
    per qb: PE-transpose -> [128, 65]; r1 = 1/col64;
      att  = (Atil_t * r1/(N+1)) + crep3   [one fused DVE op]
      c_att= (Atil_t * -r1/(N-1)) + crep2  [one fused DVE op]
"""

import sys

if "/opt/trn_rl_repo" not in sys.path:
    sys.path.insert(0, "/opt/trn_rl_repo")

from contextlib import ExitStack

import numpy as np

import concourse.bass as bass
from concourse import bacc, masks, mybir
from concourse.bass_utils import run_bass_kernel_spmd
from concourse.tile import TileContext

F32 = mybir.dt.float32
F32R = mybir.dt.float32r
BF16 = mybir.dt.bfloat16
EXP = mybir.ActivationFunctionType.Exp
MULT = mybir.AluOpType.mult
ADD = mybir.AluOpType.add
AXX = mybir.AxisListType.X

P = 128          # partitions
N = 1024         # tokens
D = 1024         # model dim
NF = 512         # features per core (8 heads x 64)
FH = 8           # heads per core
DK = 64          # head dim
NPT = N // P     # 8 token ptiles
KBN = D // P     # 8 contraction blocks
MB = NF // P     # 4 feature ptiles


def _r(ap):
    return ap.bitcast(F32R)


def build_nc():
    nc = bacc.Bacc("TRN2")
    x_d = nc.dram_tensor("x", [N, D], F32, kind="ExternalInput")
    y_d = nc.dram_tensor("y", [N, D], F32, kind="ExternalInput")
    wq_d = nc.dram_tensor("wq", [NF, D], F32, kind="ExternalInput")
    wk_d = nc.dram_tensor("wk", [NF, D], F32, kind="ExternalInput")
    wv_d = nc.dram_tensor("wv", [NF, D], F32, kind="ExternalInput")
    catt_d = nc.dram_tensor("catt", [N, NF], F32, kind="ExternalOutput")
    att_d = nc.dram_tensor("att", [N, NF], F32, kind="ExternalOutput")

    with TileContext(nc) as tc, ExitStack() as ctx:
        persist = ctx.enter_context(tc.tile_pool(name="persist", bufs=1))
        identb = persist.tile([P, P], BF16)
        identr = persist.tile([P, P], F32R)

        qt = persist.tile([P, MB, N], BF16)       # Q^T: [feat%128, featblk, tok]
        kt = persist.tile([P, MB, N], BF16)
        vv = persist.tile([P, NPT, FH, DK + 1], BF16)  # V_aug per head
        att_sb = persist.tile([P, NPT, NF], F32)
        catt_sb = persist.tile([P, NPT, NF], F32)
        crep3 = persist.tile([P, NF], F32)        # colsum(V)/(N+1), bcast over q
        crep2 = persist.tile([P, NF], F32)        # colsum(V)/(N-1)




        # ---------------- setup: transposes + projections ----------------
        with ExitStack() as sctx:
            sbp = sctx.enter_context(tc.tile_pool(name="setup", bufs=1))
            pst = sctx.enter_context(tc.tile_pool(name="pst", bufs=4, space="PSUM"))
            wp = sctx.enter_context(tc.tile_pool(name="wp", bufs=1))

            masks.make_identity(nc, identb[:])
            nc.scalar.copy(identr[:], identb[:])
            # ones columns of V_aug: single strided DVE op, out = in*0 + 1
            nc.vector.tensor_scalar(
                out=vv[:, :, :, DK:DK + 1].rearrange("p i h one -> p (i h one)"),
                in0=identb[:, 0:NPT * FH],
                scalar1=0.0, scalar2=1.0, op0=MULT, op1=ADD,
            )

            xt = sbp.tile([P, KBN, N], F32, tag="xt")
            yt = sbp.tile([P, KBN, N], F32, tag="yt")
            onescol = sbp.tile([1, P], F32, tag="ones")
            nc.scalar.activation(_r(onescol[:]), identb[0:1, :],
                                 mybir.ActivationFunctionType.Copy,
                                 bias=1.0, scale=0.0)

            # DMA queue order (single sync queue, each tensor lands just
            # before its consumer): x, Wq, Wk, y, Wv
            wraw_q = wp.tile([P, MB, D], F32R, tag="wraw_q")
            wraw_k = wp.tile([P, MB, D], F32R, tag="wraw_k")

            def _transpose_in(raw, dst, ccopy):
                for kb in range(KBN):
                    for half in range(2):
                        tp = pst.tile([P, 512], F32, tag="tp")
                        for j in range(4):
                            i = half * 4 + j
                            nc.tensor.transpose(
                                _r(tp[:, j * P:(j + 1) * P]),
                                _r(raw[:, i, kb * P:(kb + 1) * P]),
                                identr[:],
                            )
                        ccopy(
                            _r(dst[:, kb, half * 512:(half + 1) * 512]), tp[:],
                        )

            def _copy_v(out, in_):
                nc.vector.tensor_copy(out, in_)

            def _copy_s(out, in_):
                nc.scalar.copy(out, in_)

            rp = sctx.enter_context(tc.tile_pool(name="rawxy", bufs=1))
            raw_x = rp.tile([P, NPT, D], F32R, tag="raw")
            for i in range(NPT):
                nc.sync.dma_start(out=raw_x[:, i, :],
                                  in_=_r(x_d[i * P:(i + 1) * P, :]))
            for m in range(MB):
                nc.sync.dma_start(out=wraw_q[:, m, :],
                                  in_=_r(wq_d[m * P:(m + 1) * P, :]))
            for m in range(MB):
                nc.sync.dma_start(out=wraw_k[:, m, :],
                                  in_=_r(wk_d[m * P:(m + 1) * P, :]))
            _transpose_in(raw_x, xt, _copy_v)

            def load_wt(wraw):
                wt = wp.tile([P, KBN, 512], F32, tag="wt")
                for kb in range(KBN):
                    tp = pst.tile([P, 512], F32, tag="tp")
                    for m in range(MB):
                        nc.tensor.transpose(
                            _r(tp[:, m * P:(m + 1) * P]),
                            _r(wraw[:, m, kb * P:(kb + 1) * P]),
                            identr[:],
                        )
                    nc.scalar.copy(wt[:, kb, :].bitcast(F32R), tp[:])
                return wt

            for wraw, out_sb in ((wraw_q, qt), (wraw_k, kt)):
                wt = load_wt(wraw)
                for m in range(MB):
                    q_ps = pst.tile([P, N], F32, tag="proj", bufs=2)
                    for ch in range(2):
                        for kb in range(KBN):
                            nc.tensor.matmul(
                                q_ps[:, ch * 512:(ch + 1) * 512],
                                lhsT=_r(wt[:, kb, m * P:(m + 1) * P]),
                                rhs=_r(xt[:, kb, ch * 512:(ch + 1) * 512]),
                                start=(kb == 0),
                                stop=(kb == KBN - 1),
                            )
                    nc.scalar.copy(out_sb[:, m, :], q_ps[:])

            # y arrives while the q/k projections run
            raw_y = rp.tile([P, NPT, D], F32R, tag="raw")
            for i in range(NPT):
                nc.sync.dma_start(out=raw_y[:, i, :],
                                  in_=_r(y_d[i * P:(i + 1) * P, :]))
            wraw_v = wp.tile([P, MB, D], F32R, tag="wraw_q")
            for m in range(MB):
                nc.sync.dma_start(out=wraw_v[:, m, :],
                                  in_=_r(wv_d[m * P:(m + 1) * P, :]))
            _transpose_in(raw_y, yt, _copy_v)

            # colsum(y) over tokens (free-dim reduce on y^T), for C = Wv @ ysum
            ysum = sbp.tile([P, KBN], F32, tag="ysum")
            with nc.allow_low_precision(reason="f32r bitcast of f32 accumulate"):
                nc.vector.reduce_sum(out=_r(ysum[:]), in_=yt[:], axis=AXX)

            wvt = load_wt(wraw_v)
            for i in range(NPT):
                v_ps = pst.tile([P, 512], F32, tag="tp")
                for kb in range(KBN):
                    nc.tensor.matmul(
                        v_ps[:],
                        lhsT=_r(yt[:, kb, i * P:(i + 1) * P]),
                        rhs=_r(wvt[:, kb, :]),
                        start=(kb == 0),
                        stop=(kb == KBN - 1),
                    )
                nc.scalar.copy(
                    vv[:, i, :, 0:DK],
                    v_ps[:].rearrange("p (h d) -> p h d", h=FH),
                )

            # C row = Wv_s @ ysum  -> [1, 512] then broadcast to crep3/crep2
            c_tile = pst.tile([P, 512], F32, tag="tp")
            c_ps = c_tile[0:1, :]
            for kb in range(KBN):
                nc.tensor.matmul(
                    c_ps,
                    lhsT=_r(ysum[:, kb:kb + 1]),
                    rhs=_r(wvt[:, kb, :]),
                    start=(kb == 0),
                    stop=(kb == KBN - 1),
                )
            c_row = sbp.tile([1, 512], F32, tag="crow_sb")
            nc.scalar.copy(_r(c_row[:]), c_ps)
            crep_ps = pst.tile([P, 512], F32, tag="tp")
            nc.tensor.matmul(
                crep_ps[:], lhsT=_r(onescol[:]), rhs=_r(c_row[:]),
                start=True, stop=True,
            )
            nc.scalar.mul(crep3[:], crep_ps[:], 1.0 / (N + 1))
            nc.scalar.mul(crep2[:], crep_ps[:], 1.0 / (N - 1))

        # ---------------- per-head attention ----------------
        e1p = ctx.enter_context(tc.tile_pool(name="e1p", bufs=3))
        asb = ctx.enter_context(tc.tile_pool(name="asb", bufs=2))
        smp = ctx.enter_context(tc.tile_pool(name="smp", bufs=8))
        psb = ctx.enter_context(tc.tile_pool(name="psb", bufs=2, space="PSUM"))
        pa = ctx.enter_context(tc.tile_pool(name="pa", bufs=1, space="PSUM"))
        pot = ctx.enter_context(tc.tile_pool(name="pot", bufs=2, space="PSUM"))

        def epilogue(h, a_sb):
            # PE-transpose Atil^T back to [q, dv] (+ rowsum col), then the
            # fused per-q normalize + C offset on DVE
            ot = pot.tile([P, NPT, DK + 2], BF16, tag="ot")
            for qb in range(NPT):
                nc.tensor.transpose(
                    ot[:, qb, 0:DK + 1],
                    a_sb[:, qb * P:(qb + 1) * P],
                    identb[0:DK + 1, 0:DK + 1],
                )
            r1 = smp.tile([P, NPT], F32, tag="r1")
            nc.vector.reciprocal(r1[:], ot[:, :, DK])
            r1a = smp.tile([P, NPT], F32, tag="r1")
            r1b = smp.tile([P, NPT], F32, tag="r1")
            nc.vector.tensor_scalar_mul(r1a[:], r1[:], 1.0 / (N + 1))
            nc.vector.tensor_scalar_mul(r1b[:], r1[:], -1.0 / (N - 1))
            for qb in range(NPT):
                nc.vector.scalar_tensor_tensor(
                    out=att_sb[:, qb, h * DK:(h + 1) * DK],
                    in0=ot[:, qb, 0:DK],
                    scalar=r1a[:, qb:qb + 1],
                    in1=crep3[:, h * DK:(h + 1) * DK],
                    op0=MULT,
                    op1=ADD,
                )
                nc.vector.scalar_tensor_tensor(
                    out=catt_sb[:, qb, h * DK:(h + 1) * DK],
                    in0=ot[:, qb, 0:DK],
                    scalar=r1b[:, qb:qb + 1],
                    in1=crep2[:, h * DK:(h + 1) * DK],
                    op0=MULT,
                    op1=ADD,
                )

        def stream_out(h, dengs):
            # stream the finished head pair; scalar queue only joins for the
            # last pair (mid-loop enqueues on ScalarE would stall the exps)
            nd = len(dengs)
            cs = slice((h - 1) * DK, (h + 1) * DK)
            for i in range(NPT):
                dengs[(2 * i) % nd].dma_start(
                    out=att_d[i * P:(i + 1) * P, cs], in_=att_sb[:, i, cs])
                dengs[(2 * i + 1) % nd].dma_start(
                    out=catt_d[i * P:(i + 1) * P, cs], in_=catt_sb[:, i, cs])

        prev = None
        for h in range(FH):
            hb, ho = h // 2, (h % 2) * DK
            e1t = e1p.tile([P, KBN, N], BF16, tag="e1")
            for kb in range(KBN):
                s_ps = psb.tile([P, N], F32, tag="st")
                for ch in range(2):
                    nc.tensor.matmul(
                        s_ps[:, ch * 512:(ch + 1) * 512],
                        lhsT=kt[ho:ho + DK, hb, kb * P:(kb + 1) * P],
                        rhs=qt[ho:ho + DK, hb, ch * 512:(ch + 1) * 512],
                        start=True,
                        stop=True,
                    )
                nc.scalar.activation(e1t[:, kb, :], s_ps[:], EXP, scale=0.125)

            a_ps = pa.tile([DK + 1, N], F32, tag="a")
            for kb in range(KBN):
                for ch in range(2):
                    nc.tensor.matmul(
                        a_ps[:, ch * 512:(ch + 1) * 512],
                        lhsT=vv[:, kb, h, :],
                        rhs=e1t[:, kb, ch * 512:(ch + 1) * 512],
                        start=(kb == 0),
                        stop=(kb == KBN - 1),
                    )
            a_sb = asb.tile([DK + 1, N], BF16, tag="at")
            nc.vector.tensor_copy(a_sb[:], a_ps[:])
            # epilogue of the PREVIOUS head: keeps this head's a_sb copy at
            # the front of the DVE queue so the next apply is never blocked
            if prev is not None:
                epilogue(*prev)
                if prev[0] % 2 == 1:
                    stream_out(prev[0], (nc.sync, nc.gpsimd))
            prev = (h, a_sb)
        epilogue(*prev)
        stream_out(prev[0], (nc.sync, nc.gpsimd, nc.scalar))

    nc.finalize()
    return nc


_NC_CACHE = {}


def _get_nc():
    if "nc" not in _NC_CACHE:
        _NC_CACHE["nc"] = build_nc()
    return _NC_CACHE["nc"]


def _make_in_maps(x, y, Wq, Wk, Wv):
    x = np.ascontiguousarray(np.asarray(x, dtype=np.float32))
    y = np.ascontiguousarray(np.asarray(y, dtype=np.float32))
    Wq = np.ascontiguousarray(np.asarray(Wq, dtype=np.float32))
    Wk = np.ascontiguousarray(np.asarray(Wk, dtype=np.float32))
    Wv = np.ascontiguousarray(np.asarray(Wv, dtype=np.float32))
    in_maps = []
    for c in range(8):
        b, h0 = c // 2, (c % 2) * 8
        rows = slice(h0 * DK, h0 * DK + NF)
        in_maps.append({
            "x": x[b],
            "y": y[b],
            "wq": np.ascontiguousarray(Wq[rows]),
            "wk": np.ascontiguousarray(Wk[rows]),
            "wv": np.ascontiguousarray(Wv[rows]),
        })
    return in_maps


def run_cores(x, y, Wq, Wk, Wv, trace=False, tmpdir=None):
    nc = _get_nc()
    res = run_bass_kernel_spmd(
        nc, _make_in_maps(x, y, Wq, Wk, Wv), core_ids=list(range(8)),
        trace=trace, tmpdir=tmpdir,
    )
    B = 4
    c_att = np.empty((B, N, 2 * NF), dtype=np.float32)
    att = np.empty((B, N, 2 * NF), dtype=np.float32)
    for c, r in enumerate(res.results):
        b, cols = c // 2, slice((c % 2) * NF, (c % 2) * NF + NF)
        c_att[b][:, cols] = r["catt"]
        att[b][:, cols] = r["att"]
    return (c_att, att), res


def kernel(x, y, Wq, Wk, Wv):
    out, _ = run_cores(x, y, Wq, Wk, Wv)
    return out


# revision 52
# speedup vs baseline: 1.0548x; 1.0548x over previous
"""Trainium2 Bass kernel: dual-softmax ("contrast") multi-head self-attention.

Problem (per full input):
  x, y: (4, 1024, 1024) f32; Wq/Wk/Wv: (1024, 1024) f32, nh=16 heads, dk=dv=64.
  q = x @ Wq.T, k = x @ Wk.T, v = y @ Wv.T  (split heads)
  dist   = softmax(q k^T / 8)
  c_att  = softmax(1 - dist) @ v      (== softmax(-dist) @ v, shift invariance)
  att    = softmax(dist) @ v
  returns (c_att, att), each (4, 1024, 1024) f32.

Key numerics: dist entries are softmax outputs (rows sum to exactly 1, entries
in [0,1], overwhelmingly ~1e-3), so exp(+-dist) = 1 +- dist to ~5e-3 relative
output error (tolerance 2e-2; verified against the oracle on CPU). With the
linearization BOTH branches share a single apply matmul and the second-softmax
normalizers are constants:
  att   = (C + A) / (N+1),   c_att = (C - A) / (N-1)
  A = dist @ v = r1[q] * (E1 @ v), E1 = exp(S/8), r1 = 1/rowsum(E1), C = colsum(v)
Per-q scales (r1/(N+-1)) and the C offset are applied in a fused DVE epilogue
after a single PE transpose of Atilde^T = v^T-stationary @ E1^T.

Sharding: 8 cores = 4 batches x 2 head-groups (8 heads each). Each core gets
x[b], y[b] and a 512-row slice of each weight; returns (c_att, att) slices
[1024, 512].

Per-core algorithm:
  XT = x^T, YT = y^T via PE transposes (f32r).
  QT = Wq_s @ x^T   [feat, tok]  bf16 (f32r matmuls, K-accumulated in PSUM)
  KT = Wk_s @ x^T   [feat, tok]  bf16
  V  = y @ Wv_s^T   [tok, feat]  bf16, stored per head with a ones column.
  C  = Wv_s @ colsum(y) once via PE; crep3/2 = broadcast(C)/(N+-1).
  per head (all matmuls bf16, 1 cyc/row):
    S^T[kb] = KT_h[:,kb]^T-stationary @ QT_h    (k on partitions, q on free)
    E1T[kb] = exp(S^T/8)                        [ScalarE, out bf16]
    Atil^T  = sum_kb V_aug[kb]^T-stationary @ E1T[kb]  -> [65, 1024] PSUM,
              row 64 = rowsum1(q) (ones column of V_aug)
    per qb: PE-transpose -> [128, 65]; r1 = 1/col64;
      att  = (Atil_t * r1/(N+1)) + crep3   [one fused DVE op]
      c_att= (Atil_t * -r1/(N-1)) + crep2  [one fused DVE op]
"""

import sys

if "/opt/trn_rl_repo" not in sys.path:
    sys.path.insert(0, "/opt/trn_rl_repo")

from contextlib import ExitStack

import numpy as np

import concourse.bass as bass
from concourse import bacc, masks, mybir
from concourse.bass_utils import run_bass_kernel_spmd
from concourse.tile import TileContext

F32 = mybir.dt.float32
F32R = mybir.dt.float32r
BF16 = mybir.dt.bfloat16
EXP = mybir.ActivationFunctionType.Exp
MULT = mybir.AluOpType.mult
ADD = mybir.AluOpType.add
AXX = mybir.AxisListType.X

P = 128          # partitions
N = 1024         # tokens
D = 1024         # model dim
NF = 512         # features per core (8 heads x 64)
FH = 8           # heads per core
DK = 64          # head dim
NPT = N // P     # 8 token ptiles
KBN = D // P     # 8 contraction blocks
MB = NF // P     # 4 feature ptiles


def _r(ap):
    return ap.bitcast(F32R)


def build_nc():
    nc = bacc.Bacc("TRN2")
    x_d = nc.dram_tensor("x", [N, D], F32, kind="ExternalInput")
    y_d = nc.dram_tensor("y", [N, D], F32, kind="ExternalInput")
    wq_d = nc.dram_tensor("wq", [NF, D], F32, kind="ExternalInput")
    wk_d = nc.dram_tensor("wk", [NF, D], F32, kind="ExternalInput")
    wv_d = nc.dram_tensor("wv", [NF, D], F32, kind="ExternalInput")
    catt_d = nc.dram_tensor("catt", [N, NF], F32, kind="ExternalOutput")
    att_d = nc.dram_tensor("att", [N, NF], F32, kind="ExternalOutput")

    with TileContext(nc) as tc, ExitStack() as ctx:
        persist = ctx.enter_context(tc.tile_pool(name="persist", bufs=1))
        identb = persist.tile([P, P], BF16)
        identr = persist.tile([P, P], F32R)

        qt = persist.tile([P, MB, N], BF16)       # Q^T: [feat%128, featblk, tok]
        kt = persist.tile([P, MB, N], BF16)
        vv = persist.tile([P, NPT, FH, DK + 1], BF16)  # V_aug per head
        att_sb = persist.tile([P, NPT, NF], F32)
        catt_sb = persist.tile([P, NPT, NF], F32)
        crep3 = persist.tile([P, NF], F32)        # colsum(V)/(N+1), bcast over q
        crep2 = persist.tile([P, NF], F32)        # colsum(V)/(N-1)




        # ---------------- setup: transposes + projections ----------------
        with ExitStack() as sctx:
            sbp = sctx.enter_context(tc.tile_pool(name="setup", bufs=1))
            pst = sctx.enter_context(tc.tile_pool(name="pst", bufs=4, space="PSUM"))
            wp = sctx.enter_context(tc.tile_pool(name="wp", bufs=1))

            masks.make_identity(nc, identb[:])
            nc.scalar.copy(identr[:], identb[:])
            # ones columns of V_aug: single strided DVE op, out = in*0 + 1
            nc.vector.tensor_scalar(
                out=vv[:, :, :, DK:DK + 1].rearrange("p i h one -> p (i h one)"),
                in0=identb[:, 0:NPT * FH],
                scalar1=0.0, scalar2=1.0, op0=MULT, op1=ADD,
            )

            xt = sbp.tile([P, KBN, N], F32, tag="xt")
            yt = sbp.tile([P, KBN, N], F32, tag="yt")
            onescol = sbp.tile([1, P], F32, tag="ones")
            nc.scalar.activation(_r(onescol[:]), identb[0:1, :],
                                 mybir.ActivationFunctionType.Copy,
                                 bias=1.0, scale=0.0)

            # DMA queue order (single sync queue, each tensor lands just
            # before its consumer): x, Wq, Wk, y, Wv
            wraw_q = wp.tile([P, MB, D], F32R, tag="wraw_q")
            wraw_k = wp.tile([P, MB, D], F32R, tag="wraw_k")

            def _transpose_in(raw, dst, ccopy):
                for kb in range(KBN):
                    for half in range(2):
                        tp = pst.tile([P, 512], F32, tag="tp")
                        for j in range(4):
                            i = half * 4 + j
                            nc.tensor.transpose(
                                tp[:, j * P:(j + 1) * P],
                                raw[:, i, kb * P:(kb + 1) * P].bitcast(F32),
                                identr[:].bitcast(F32),
                            )
                        ccopy(
                            _r(dst[:, kb, half * 512:(half + 1) * 512]), tp[:],
                        )

            def _copy_v(out, in_):
                nc.vector.tensor_copy(out, in_)

            def _copy_s(out, in_):
                nc.scalar.copy(out, in_)

            rp = sctx.enter_context(tc.tile_pool(name="rawxy", bufs=1))
            raw_x = rp.tile([P, NPT, D], F32R, tag="raw")
            for i in range(NPT):
                nc.sync.dma_start(out=raw_x[:, i, :],
                                  in_=_r(x_d[i * P:(i + 1) * P, :]))
            for m in range(MB):
                nc.sync.dma_start(out=wraw_q[:, m, :],
                                  in_=_r(wq_d[m * P:(m + 1) * P, :]))
            for m in range(MB):
                nc.sync.dma_start(out=wraw_k[:, m, :],
                                  in_=_r(wk_d[m * P:(m + 1) * P, :]))
            _transpose_in(raw_x, xt, _copy_v)

            def load_wt(wraw):
                wt = wp.tile([P, KBN, 512], F32, tag="wt")
                for kb in range(KBN):
                    tp = pst.tile([P, 512], F32, tag="tp")
                    for m in range(MB):
                        nc.tensor.transpose(
                            tp[:, m * P:(m + 1) * P],
                            wraw[:, m, kb * P:(kb + 1) * P].bitcast(F32),
                            identr[:].bitcast(F32),
                        )
                    nc.scalar.copy(wt[:, kb, :].bitcast(F32R), tp[:])
                return wt

            for wraw, out_sb in ((wraw_q, qt), (wraw_k, kt)):
                wt = load_wt(wraw)
                for m in range(MB):
                    q_ps = pst.tile([P, N], F32, tag="proj", bufs=2)
                    for ch in range(2):
                        for kb in range(KBN):
                            nc.tensor.matmul(
                                q_ps[:, ch * 512:(ch + 1) * 512],
                                lhsT=_r(wt[:, kb, m * P:(m + 1) * P]),
                                rhs=_r(xt[:, kb, ch * 512:(ch + 1) * 512]),
                                start=(kb == 0),
                                stop=(kb == KBN - 1),
                            )
                    nc.scalar.copy(out_sb[:, m, :], q_ps[:])

            # y arrives while the q/k projections run
            raw_y = rp.tile([P, NPT, D], F32R, tag="raw")
            for i in range(NPT):
                nc.sync.dma_start(out=raw_y[:, i, :],
                                  in_=_r(y_d[i * P:(i + 1) * P, :]))
            wraw_v = wp.tile([P, MB, D], F32R, tag="wraw_q")
            for m in range(MB):
                nc.sync.dma_start(out=wraw_v[:, m, :],
                                  in_=_r(wv_d[m * P:(m + 1) * P, :]))
            _transpose_in(raw_y, yt, _copy_v)

            # colsum(y) over tokens for C = Wv @ ysum — on ScalarE (accum_out
            # of copies into dead raw_y scratch) so the DVE queue stays clear
            # for the head-0 epilogue chain
            ysum = sbp.tile([P, KBN], F32, tag="ysum")
            with nc.allow_low_precision(reason="f32r bitcast of f32 accumulate"):
                for kb in range(KBN):
                    nc.scalar.activation(
                        raw_y[:, kb, :].bitcast(F32), yt[:, kb, :],
                        mybir.ActivationFunctionType.Copy,
                        accum_out=_r(ysum[:, kb:kb + 1]),
                    )

            wvt = load_wt(wraw_v)
            for i in range(NPT):
                v_ps = pst.tile([P, 512], F32, tag="tp")
                for kb in range(KBN):
                    nc.tensor.matmul(
                        v_ps[:],
                        lhsT=_r(yt[:, kb, i * P:(i + 1) * P]),
                        rhs=_r(wvt[:, kb, :]),
                        start=(kb == 0),
                        stop=(kb == KBN - 1),
                    )
                nc.scalar.copy(
                    vv[:, i, :, 0:DK],
                    v_ps[:].rearrange("p (h d) -> p h d", h=FH),
                )

            # C row = Wv_s @ ysum  -> [1, 512] then broadcast to crep3/crep2
            c_tile = pst.tile([P, 512], F32, tag="tp")
            c_ps = c_tile[0:1, :]
            for kb in range(KBN):
                nc.tensor.matmul(
                    c_ps,
                    lhsT=_r(ysum[:, kb:kb + 1]),
                    rhs=_r(wvt[:, kb, :]),
                    start=(kb == 0),
                    stop=(kb == KBN - 1),
                )
            c_row = sbp.tile([1, 512], F32, tag="crow_sb")
            nc.scalar.copy(_r(c_row[:]), c_ps)
            crep_ps = pst.tile([P, 512], F32, tag="tp")
            nc.tensor.matmul(
                crep_ps[:], lhsT=_r(onescol[:]), rhs=_r(c_row[:]),
                start=True, stop=True,
            )
            nc.scalar.mul(crep3[:], crep_ps[:], 1.0 / (N + 1))
            nc.scalar.mul(crep2[:], crep_ps[:], 1.0 / (N - 1))

        # ---------------- per-head attention ----------------
        e1p = ctx.enter_context(tc.tile_pool(name="e1p", bufs=3))
        asb = ctx.enter_context(tc.tile_pool(name="asb", bufs=2))
        smp = ctx.enter_context(tc.tile_pool(name="smp", bufs=8))
        psb = ctx.enter_context(tc.tile_pool(name="psb", bufs=2, space="PSUM"))
        pa = ctx.enter_context(tc.tile_pool(name="pa", bufs=1, space="PSUM"))
        pot = ctx.enter_context(tc.tile_pool(name="pot", bufs=2, space="PSUM"))

        def epilogue(h, a_sb):
            # PE-transpose Atil^T back to [q, dv] (+ rowsum col), then the
            # fused per-q normalize + C offset on DVE
            ot = pot.tile([P, NPT, DK + 2], BF16, tag="ot")
            for qb in range(NPT):
                nc.tensor.transpose(
                    ot[:, qb, 0:DK + 1],
                    a_sb[:, qb * P:(qb + 1) * P],
                    identb[0:DK + 1, 0:DK + 1],
                )
            r1 = smp.tile([P, NPT], F32, tag="r1")
            nc.vector.reciprocal(r1[:], ot[:, :, DK])
            r1a = smp.tile([P, NPT], F32, tag="r1")
            r1b = smp.tile([P, NPT], F32, tag="r1")
            nc.vector.tensor_scalar_mul(r1a[:], r1[:], 1.0 / (N + 1))
            nc.vector.tensor_scalar_mul(r1b[:], r1[:], -1.0 / (N - 1))
            for qb in range(NPT):
                nc.vector.scalar_tensor_tensor(
                    out=att_sb[:, qb, h * DK:(h + 1) * DK],
                    in0=ot[:, qb, 0:DK],
                    scalar=r1a[:, qb:qb + 1],
                    in1=crep3[:, h * DK:(h + 1) * DK],
                    op0=MULT,
                    op1=ADD,
                )
                nc.vector.scalar_tensor_tensor(
                    out=catt_sb[:, qb, h * DK:(h + 1) * DK],
                    in0=ot[:, qb, 0:DK],
                    scalar=r1b[:, qb:qb + 1],
                    in1=crep2[:, h * DK:(h + 1) * DK],
                    op0=MULT,
                    op1=ADD,
                )

        def stream_out(h, dengs):
            # stream the finished head pair; scalar queue only joins for the
            # last pair (mid-loop enqueues on ScalarE would stall the exps)
            nd = len(dengs)
            cs = slice((h - 1) * DK, (h + 1) * DK)
            for i in range(NPT):
                dengs[(2 * i) % nd].dma_start(
                    out=att_d[i * P:(i + 1) * P, cs], in_=att_sb[:, i, cs])
                dengs[(2 * i + 1) % nd].dma_start(
                    out=catt_d[i * P:(i + 1) * P, cs], in_=catt_sb[:, i, cs])

        prev = None
        for h in range(FH):
            hb, ho = h // 2, (h % 2) * DK
            e1t = e1p.tile([P, KBN, N], BF16, tag="e1")
            for kb in range(KBN):
                s_ps = psb.tile([P, N], F32, tag="st")
                for ch in range(2):
                    nc.tensor.matmul(
                        s_ps[:, ch * 512:(ch + 1) * 512],
                        lhsT=kt[ho:ho + DK, hb, kb * P:(kb + 1) * P],
                        rhs=qt[ho:ho + DK, hb, ch * 512:(ch + 1) * 512],
                        start=True,
                        stop=True,
                    )
                nc.scalar.activation(e1t[:, kb, :], s_ps[:], EXP, scale=0.125)

            a_ps = pa.tile([DK + 1, N], F32, tag="a")
            for kb in range(KBN):
                for ch in range(2):
                    nc.tensor.matmul(
                        a_ps[:, ch * 512:(ch + 1) * 512],
                        lhsT=vv[:, kb, h, :],
                        rhs=e1t[:, kb, ch * 512:(ch + 1) * 512],
                        start=(kb == 0),
                        stop=(kb == KBN - 1),
                    )
            a_sb = asb.tile([DK + 1, N], BF16, tag="at")
            nc.vector.tensor_copy(a_sb[:], a_ps[:])
            # epilogue of the PREVIOUS head: keeps this head's a_sb copy at
            # the front of the DVE queue so the next apply is never blocked
            if prev is not None:
                epilogue(*prev)
                if prev[0] % 2 == 1:
                    stream_out(prev[0], (nc.sync, nc.gpsimd))
            prev = (h, a_sb)
        epilogue(*prev)
        stream_out(prev[0], (nc.sync, nc.gpsimd, nc.scalar))

    nc.finalize()
    return nc


_NC_CACHE = {}


def _get_nc():
    if "nc" not in _NC_CACHE:
        _NC_CACHE["nc"] = build_nc()
    return _NC_CACHE["nc"]


def _make_in_maps(x, y, Wq, Wk, Wv):
    x = np.ascontiguousarray(np.asarray(x, dtype=np.float32))
    y = np.ascontiguousarray(np.asarray(y, dtype=np.float32))
    Wq = np.ascontiguousarray(np.asarray(Wq, dtype=np.float32))
    Wk = np.ascontiguousarray(np.asarray(Wk, dtype=np.float32))
    Wv = np.ascontiguousarray(np.asarray(Wv, dtype=np.float32))
    in_maps = []
    for c in range(8):
        b, h0 = c // 2, (c % 2) * 8
        rows = slice(h0 * DK, h0 * DK + NF)
        in_maps.append({
            "x": x[b],
            "y": y[b],
            "wq": np.ascontiguousarray(Wq[rows]),
            "wk": np.ascontiguousarray(Wk[rows]),
            "wv": np.ascontiguousarray(Wv[rows]),
        })
    return in_maps


def run_cores(x, y, Wq, Wk, Wv, trace=False, tmpdir=None):
    nc = _get_nc()
    res = run_bass_kernel_spmd(
        nc, _make_in_maps(x, y, Wq, Wk, Wv), core_ids=list(range(8)),
        trace=trace, tmpdir=tmpdir,
    )
    B = 4
    c_att = np.empty((B, N, 2 * NF), dtype=np.float32)
    att = np.empty((B, N, 2 * NF), dtype=np.float32)
    for c, r in enumerate(res.results):
        b, cols = c // 2, slice((c % 2) * NF, (c % 2) * NF + NF)
        c_att[b][:, cols] = r["catt"]
        att[b][:, cols] = r["att"]
    return (c_att, att), res


def kernel(x, y, Wq, Wk, Wv):
    out, _ = run_cores(x, y, Wq, Wk, Wv)
    return out


# revision 54
# speedup vs baseline: 1.1117x; 1.0539x over previous
"""Trainium2 Bass kernel: dual-softmax ("contrast") multi-head self-attention.

Problem (per full input):
  x, y: (4, 1024, 1024) f32; Wq/Wk/Wv: (1024, 1024) f32, nh=16 heads, dk=dv=64.
  q = x @ Wq.T, k = x @ Wk.T, v = y @ Wv.T  (split heads)
  dist   = softmax(q k^T / 8)
  c_att  = softmax(1 - dist) @ v      (== softmax(-dist) @ v, shift invariance)
  att    = softmax(dist) @ v
  returns (c_att, att), each (4, 1024, 1024) f32.

Key numerics: dist entries are softmax outputs (rows sum to exactly 1, entries
in [0,1], overwhelmingly ~1e-3), so exp(+-dist) = 1 +- dist to ~5e-3 relative
output error (tolerance 2e-2; verified against the oracle on CPU). With the
linearization BOTH branches share a single apply matmul and the second-softmax
normalizers are constants:
  att   = (C + A) / (N+1),   c_att = (C - A) / (N-1)
  A = dist @ v = r1[q] * (E1 @ v), E1 = exp(S/8), r1 = 1/rowsum(E1), C = colsum(v)
Per-q scales (r1/(N+-1)) and the C offset are applied in a fused DVE epilogue
after a single PE transpose of Atilde^T = v^T-stationary @ E1^T.

Sharding: 8 cores = 4 batches x 2 head-groups (8 heads each). Each core gets
x[b], y[b] and a 512-row slice of each weight; returns (c_att, att) slices
[1024, 512].

Per-core algorithm:
  XT = x^T, YT = y^T via PE transposes (f32r).
  QT = Wq_s @ x^T   [feat, tok]  bf16 (f32r matmuls, K-accumulated in PSUM)
  KT = Wk_s @ x^T   [feat, tok]  bf16
  V  = y @ Wv_s^T   [tok, feat]  bf16, stored per head with a ones column.
  C  = Wv_s @ colsum(y) once via PE; crep3/2 = broadcast(C)/(N+-1).
  per head (all matmuls bf16, 1 cyc/row):
    S^T[kb] = KT_h[:,kb]^T-stationary @ QT_h    (k on partitions, q on free)
    E1T[kb] = exp(S^T/8)                        [ScalarE, out bf16]
    Atil^T  = sum_kb V_aug[kb]^T-stationary @ E1T[kb]  -> [65, 1024] PSUM,
              row 64 = rowsum1(q) (ones column of V_aug)
    per qb: PE-transpose -> [128, 65]; r1 = 1/col64;
      att  = (Atil_t * r1/(N+1)) + crep3   [one fused DVE op]
      c_att= (Atil_t * -r1/(N-1)) + crep2  [one fused DVE op]
"""

import sys

if "/opt/trn_rl_repo" not in sys.path:
    sys.path.insert(0, "/opt/trn_rl_repo")

from contextlib import ExitStack

import numpy as np

import concourse.bass as bass
from concourse import bacc, masks, mybir
from concourse.bass_utils import run_bass_kernel_spmd
from concourse.tile import TileContext

F32 = mybir.dt.float32
F32R = mybir.dt.float32r
BF16 = mybir.dt.bfloat16
EXP = mybir.ActivationFunctionType.Exp
MULT = mybir.AluOpType.mult
ADD = mybir.AluOpType.add
AXX = mybir.AxisListType.X

P = 128          # partitions
N = 1024         # tokens
D = 1024         # model dim
NF = 512         # features per core (8 heads x 64)
FH = 8           # heads per core
DK = 64          # head dim
NPT = N // P     # 8 token ptiles
KBN = D // P     # 8 contraction blocks
MB = NF // P     # 4 feature ptiles


def _r(ap):
    return ap.bitcast(F32R)


def build_nc():
    nc = bacc.Bacc("TRN2")
    x_d = nc.dram_tensor("x", [N, D], F32, kind="ExternalInput")
    y_d = nc.dram_tensor("y", [N, D], F32, kind="ExternalInput")
    wq_d = nc.dram_tensor("wq", [NF, D], F32, kind="ExternalInput")
    wk_d = nc.dram_tensor("wk", [NF, D], F32, kind="ExternalInput")
    wv_d = nc.dram_tensor("wv", [NF, D], F32, kind="ExternalInput")
    catt_d = nc.dram_tensor("catt", [N, NF], F32, kind="ExternalOutput")
    att_d = nc.dram_tensor("att", [N, NF], F32, kind="ExternalOutput")

    with TileContext(nc) as tc, ExitStack() as ctx:
        persist = ctx.enter_context(tc.tile_pool(name="persist", bufs=1))
        identb = persist.tile([P, P], BF16)
        identr = persist.tile([P, P], F32R)

        qt = persist.tile([P, MB, N], BF16)       # Q^T: [feat%128, featblk, tok]
        kt = persist.tile([P, MB, N], BF16)
        vv = persist.tile([P, NPT, FH, DK + 1], BF16)  # V_aug per head
        att_sb = persist.tile([P, NPT, NF], F32)
        catt_sb = persist.tile([P, NPT, NF], F32)
        crep3 = persist.tile([P, NF], F32)        # colsum(V)/(N+1), bcast over q
        crep2 = persist.tile([P, NF], F32)        # colsum(V)/(N-1)




        # ---------------- setup: transposes + projections ----------------
        with ExitStack() as sctx:
            sbp = sctx.enter_context(tc.tile_pool(name="setup", bufs=1))
            pst = sctx.enter_context(tc.tile_pool(name="pst", bufs=4, space="PSUM"))
            wp = sctx.enter_context(tc.tile_pool(name="wp", bufs=1))

            masks.make_identity(nc, identb[:])
            nc.scalar.copy(identr[:], identb[:])
            # ones columns of V_aug: single strided DVE op, out = in*0 + 1
            nc.vector.tensor_scalar(
                out=vv[:, :, :, DK:DK + 1].rearrange("p i h one -> p (i h one)"),
                in0=identb[:, 0:NPT * FH],
                scalar1=0.0, scalar2=1.0, op0=MULT, op1=ADD,
            )

            xt = sbp.tile([P, KBN, N], F32, tag="xt")
            yt = sbp.tile([P, KBN, N], F32, tag="yt")
            onescol = sbp.tile([1, P], F32, tag="ones")
            nc.scalar.activation(_r(onescol[:]), identb[0:1, :],
                                 mybir.ActivationFunctionType.Copy,
                                 bias=1.0, scale=0.0)

            # DMA queue order (single sync queue, each tensor lands just
            # before its consumer): x, Wq, Wk, y, Wv
            wraw_q = wp.tile([P, MB, D], F32R, tag="wraw_q")
            wraw_k = wp.tile([P, MB, D], F32R, tag="wraw_k")

            def _transpose_in(raw, dst, ccopy):
                for kb in range(KBN):
                    for half in range(2):
                        tp = pst.tile([P, 512], F32, tag="tp")
                        for j in range(4):
                            i = half * 4 + j
                            nc.tensor.transpose(
                                tp[:, j * P:(j + 1) * P],
                                raw[:, i, kb * P:(kb + 1) * P].bitcast(F32),
                                identr[:].bitcast(F32),
                            )
                        ccopy(
                            _r(dst[:, kb, half * 512:(half + 1) * 512]), tp[:],
                        )

            def _copy_v(out, in_):
                nc.vector.tensor_copy(out, in_)

            def _copy_s(out, in_):
                nc.scalar.copy(out, in_)

            rp = sctx.enter_context(tc.tile_pool(name="rawxy", bufs=1))
            raw_x = rp.tile([P, NPT, D], F32R, tag="raw")
            for i in range(NPT):
                nc.sync.dma_start(out=raw_x[:, i, :],
                                  in_=_r(x_d[i * P:(i + 1) * P, :]))
            for m in range(MB):
                nc.sync.dma_start(out=wraw_q[:, m, :],
                                  in_=_r(wq_d[m * P:(m + 1) * P, :]))
            for m in range(MB):
                nc.sync.dma_start(out=wraw_k[:, m, :],
                                  in_=_r(wk_d[m * P:(m + 1) * P, :]))
            _transpose_in(raw_x, xt, _copy_v)

            def load_wt(wraw):
                wt = wp.tile([P, KBN, 512], F32, tag="wt")
                for kb in range(KBN):
                    tp = pst.tile([P, 512], F32, tag="tp")
                    for m in range(MB):
                        nc.tensor.transpose(
                            tp[:, m * P:(m + 1) * P],
                            wraw[:, m, kb * P:(kb + 1) * P].bitcast(F32),
                            identr[:].bitcast(F32),
                        )
                    nc.scalar.copy(wt[:, kb, :].bitcast(F32R), tp[:])
                return wt

            for wraw, out_sb in ((wraw_q, qt), (wraw_k, kt)):
                wt = load_wt(wraw)
                for m in range(MB):
                    q_ps = pst.tile([P, N], F32, tag="proj", bufs=2)
                    for ch in range(2):
                        for kb in range(KBN):
                            nc.tensor.matmul(
                                q_ps[:, ch * 512:(ch + 1) * 512],
                                lhsT=_r(wt[:, kb, m * P:(m + 1) * P]),
                                rhs=_r(xt[:, kb, ch * 512:(ch + 1) * 512]),
                                start=(kb == 0),
                                stop=(kb == KBN - 1),
                            )
                    nc.scalar.copy(out_sb[:, m, :], q_ps[:])

            # y arrives while the q/k projections run
            raw_y = rp.tile([P, NPT, D], F32R, tag="raw")
            for i in range(NPT):
                nc.sync.dma_start(out=raw_y[:, i, :],
                                  in_=_r(y_d[i * P:(i + 1) * P, :]))
            wraw_v = wp.tile([P, MB, D], F32R, tag="wraw_q")
            for m in range(MB):
                nc.sync.dma_start(out=wraw_v[:, m, :],
                                  in_=_r(wv_d[m * P:(m + 1) * P, :]))
            _transpose_in(raw_y, yt, _copy_v)

            # colsum(y) over tokens (free-dim reduce on y^T), for C = Wv @ ysum
            ysum = sbp.tile([P, KBN], F32, tag="ysum")
            with nc.allow_low_precision(reason="f32r bitcast of f32 accumulate"):
                nc.vector.reduce_sum(out=_r(ysum[:]), in_=yt[:], axis=AXX)

            wvt = load_wt(wraw_v)
            for i in range(NPT):
                v_ps = pst.tile([P, 512], F32, tag="tp")
                for kb in range(KBN):
                    nc.tensor.matmul(
                        v_ps[:],
                        lhsT=_r(yt[:, kb, i * P:(i + 1) * P]),
                        rhs=_r(wvt[:, kb, :]),
                        start=(kb == 0),
                        stop=(kb == KBN - 1),
                    )
                nc.scalar.copy(
                    vv[:, i, :, 0:DK],
                    v_ps[:].rearrange("p (h d) -> p h d", h=FH),
                )

            # C row = Wv_s @ ysum  -> [1, 512] then broadcast to crep3/crep2
            c_tile = pst.tile([P, 512], F32, tag="tp")
            c_ps = c_tile[0:1, :]
            for kb in range(KBN):
                nc.tensor.matmul(
                    c_ps,
                    lhsT=_r(ysum[:, kb:kb + 1]),
                    rhs=_r(wvt[:, kb, :]),
                    start=(kb == 0),
                    stop=(kb == KBN - 1),
                )
            c_row = sbp.tile([1, 512], F32, tag="crow_sb")
            nc.scalar.copy(_r(c_row[:]), c_ps)
            crep_ps = pst.tile([P, 512], F32, tag="tp")
            nc.tensor.matmul(
                crep_ps[:], lhsT=_r(onescol[:]), rhs=_r(c_row[:]),
                start=True, stop=True,
            )
            nc.scalar.mul(crep3[:], crep_ps[:], 1.0 / (N + 1))
            nc.scalar.mul(crep2[:], crep_ps[:], 1.0 / (N - 1))

        # ---------------- per-head attention ----------------
        e1p = ctx.enter_context(tc.tile_pool(name="e1p", bufs=2))
        asb = ctx.enter_context(tc.tile_pool(name="asb", bufs=2))
        smp = ctx.enter_context(tc.tile_pool(name="smp", bufs=8))
        psb = ctx.enter_context(tc.tile_pool(name="psb", bufs=2, space="PSUM"))
        pa = ctx.enter_context(tc.tile_pool(name="pa", bufs=1, space="PSUM"))
        pot = ctx.enter_context(tc.tile_pool(name="pot", bufs=2, space="PSUM"))

        def epilogue(h, a_sb):
            # PE-transpose Atil^T back to [q, dv] (+ rowsum col), then the
            # fused per-q normalize + C offset on DVE
            ot = pot.tile([P, NPT, DK + 2], BF16, tag="ot")
            for qb in range(NPT):
                nc.tensor.transpose(
                    ot[:, qb, 0:DK + 1],
                    a_sb[:, qb * P:(qb + 1) * P],
                    identb[0:DK + 1, 0:DK + 1],
                )
            r1 = smp.tile([P, NPT], F32, tag="r1")
            nc.vector.reciprocal(r1[:], ot[:, :, DK])
            r1a = smp.tile([P, NPT], F32, tag="r1")
            r1b = smp.tile([P, NPT], F32, tag="r1")
            nc.vector.tensor_scalar_mul(r1a[:], r1[:], 1.0 / (N + 1))
            nc.vector.tensor_scalar_mul(r1b[:], r1[:], -1.0 / (N - 1))
            for qb in range(NPT):
                nc.vector.scalar_tensor_tensor(
                    out=att_sb[:, qb, h * DK:(h + 1) * DK],
                    in0=ot[:, qb, 0:DK],
                    scalar=r1a[:, qb:qb + 1],
                    in1=crep3[:, h * DK:(h + 1) * DK],
                    op0=MULT,
                    op1=ADD,
                )
                nc.vector.scalar_tensor_tensor(
                    out=catt_sb[:, qb, h * DK:(h + 1) * DK],
                    in0=ot[:, qb, 0:DK],
                    scalar=r1b[:, qb:qb + 1],
                    in1=crep2[:, h * DK:(h + 1) * DK],
                    op0=MULT,
                    op1=ADD,
                )

        def stream_out(h, dengs):
            # stream the finished head pair; scalar queue only joins for the
            # last pair (mid-loop enqueues on ScalarE would stall the exps)
            nd = len(dengs)
            cs = slice((h - 1) * DK, (h + 1) * DK)
            for i in range(NPT):
                dengs[(2 * i) % nd].dma_start(
                    out=att_d[i * P:(i + 1) * P, cs], in_=att_sb[:, i, cs])
                dengs[(2 * i + 1) % nd].dma_start(
                    out=catt_d[i * P:(i + 1) * P, cs], in_=catt_sb[:, i, cs])

        prev = None
        for h in range(FH):
            hb, ho = h // 2, (h % 2) * DK
            e1t = e1p.tile([P, KBN, N], BF16, tag="e1")
            for kb in range(KBN):
                s_ps = psb.tile([P, N], F32, tag="st")
                for ch in range(2):
                    nc.tensor.matmul(
                        s_ps[:, ch * 512:(ch + 1) * 512],
                        lhsT=kt[ho:ho + DK, hb, kb * P:(kb + 1) * P],
                        rhs=qt[ho:ho + DK, hb, ch * 512:(ch + 1) * 512],
                        start=True,
                        stop=True,
                    )
                nc.scalar.activation(e1t[:, kb, :], s_ps[:], EXP, scale=0.125)

            a_ps = pa.tile([DK + 1, N], F32, tag="a")
            for kb in range(KBN):
                for ch in range(2):
                    nc.tensor.matmul(
                        a_ps[:, ch * 512:(ch + 1) * 512],
                        lhsT=vv[:, kb, h, :],
                        rhs=e1t[:, kb, ch * 512:(ch + 1) * 512],
                        start=(kb == 0),
                        stop=(kb == KBN - 1),
                    )
            a_sb = asb.tile([DK + 1, N], BF16, tag="at")
            nc.vector.tensor_copy(a_sb[:], a_ps[:])
            # epilogue of the PREVIOUS head: keeps this head's a_sb copy at
            # the front of the DVE queue so the next apply is never blocked
            if prev is not None:
                epilogue(*prev)
                if prev[0] % 2 == 1:
                    stream_out(prev[0], (nc.sync, nc.gpsimd))
            prev = (h, a_sb)
        epilogue(*prev)
        stream_out(prev[0], (nc.sync, nc.gpsimd, nc.scalar))

    nc.finalize()
    return nc


_NC_CACHE = {}


def _get_nc():
    if "nc" not in _NC_CACHE:
        _NC_CACHE["nc"] = build_nc()
    return _NC_CACHE["nc"]


def _make_in_maps(x, y, Wq, Wk, Wv):
    x = np.ascontiguousarray(np.asarray(x, dtype=np.float32))
    y = np.ascontiguousarray(np.asarray(y, dtype=np.float32))
    Wq = np.ascontiguousarray(np.asarray(Wq, dtype=np.float32))
    Wk = np.ascontiguousarray(np.asarray(Wk, dtype=np.float32))
    Wv = np.ascontiguousarray(np.asarray(Wv, dtype=np.float32))
    in_maps = []
    for c in range(8):
        b, h0 = c // 2, (c % 2) * 8
        rows = slice(h0 * DK, h0 * DK + NF)
        in_maps.append({
            "x": x[b],
            "y": y[b],
            "wq": np.ascontiguousarray(Wq[rows]),
            "wk": np.ascontiguousarray(Wk[rows]),
            "wv": np.ascontiguousarray(Wv[rows]),
        })
    return in_maps


def run_cores(x, y, Wq, Wk, Wv, trace=False, tmpdir=None):
    nc = _get_nc()
    res = run_bass_kernel_spmd(
        nc, _make_in_maps(x, y, Wq, Wk, Wv), core_ids=list(range(8)),
        trace=trace, tmpdir=tmpdir,
    )
    B = 4
    c_att = np.empty((B, N, 2 * NF), dtype=np.float32)
    att = np.empty((B, N, 2 * NF), dtype=np.float32)
    for c, r in enumerate(res.results):
        b, cols = c // 2, slice((c % 2) * NF, (c % 2) * NF + NF)
        c_att[b][:, cols] = r["catt"]
        att[b][:, cols] = r["att"]
    return (c_att, att), res


def kernel(x, y, Wq, Wk, Wv):
    out, _ = run_cores(x, y, Wq, Wk, Wv)
    return out
